# revision 99
# baseline (speedup 1.0000x reference)
"""Trainium2 Bass kernel for nn_Block_16621523436203 (Mamba-style block).

Sharding: pure data-parallel — batch B=8, one batch element per NeuronCore,
no collectives.  Weights are preprocessed (transposed / LN-folded / cast) on
host; each core runs the full block for its batch element.

Engine plan (per core).  HW constraints found the hard way: GPSIMD (Pool)
cannot touch PSUM and only runs plain TensorTensor (0.42 eff) + DMA;
tensor_tensor_scan is DVE-only; engine outputs consumed by f32r matmuls
must be written with f32r out-dtype; Memset cannot write f32r.

  P1  LN1 -> in_proj -> dwconv+SiLU (x and z) -> x_proj -> dt_proj/softplus
      PE: matmuls+transposes (bf16 weights), ACT: rsqrt(=Sqrt+recip)/SiLU/
      softplus (Exps then Lns batched for table locality), DVE: LN stats,
      evacuations, du mult.  z branch is emitted last so it overlaps the
      scan startup.  Weight DMAs are ordered by first use behind xin on the
      SP HWDGE queue.
  P2  selective scan, 64 groups of (8ch x 16st) partitions, processed in
      pairs ([128, 2048] tiles amortize fixed op cost):
      PE: delta-broadcast matmul (f32r) + D*u diag matmul + n-reduction
      matmul, ACT: exp(dA) + y_cm psum evac, DMA: du broadcast,
      Pool: dBu = du_bc*B mults, DVE: all scans + yt = hs*C mults.
      fc1/fc2 weights stream in via the ACT HWDGE queue during the scan.
  P3  out_proj -> +x -> LN2 -> (transpose -> fc1+GELU -> fc2 + residual)
      pipelined in L-halves; PE-bound.
ACT function-table thrash is minimized (Sqrt/Silu/Exp/Ln/Gelu runs).
"""

import sys

sys.path.insert(0, "/opt/trn_rl_repo")

import os

import ml_dtypes
import numpy as np

import concourse.bacc as bacc
import concourse.bass as bass
import concourse.mybir as mybir
import concourse.tile as tile

F32 = mybir.dt.float32
F32R = mybir.dt.float32r
BF16 = mybir.dt.bfloat16
AF = mybir.ActivationFunctionType
ALU = mybir.AluOpType

B, L, D = 8, 1024, 512
E = 1024  # d_inner
D2 = 512  # per-branch channels
R = 32  # dt_rank
NS = 16  # d_state
KC = 4  # conv kernel size
H = 2048  # mlp hidden
NCORES = 8
TT = L // 128  # 8 token tiles
DC = D // 128  # 4 d_model chunks
D2T = D2 // 128  # 4 channel tiles
ET = E // 128  # 8 d_inner tiles
HT = H // 128  # 16 hidden tiles
NG = 64  # scan groups: each = 8 channels x 16 states
EPS = 1e-5

_BF = ml_dtypes.bfloat16


def _f32r(ap):
    return ap.bitcast(F32R)


STOP_AFTER = int(os.environ.get("KSTOP", "3"))
KREPEAT = int(os.environ.get("KREPEAT", "1"))
KALLOC = int(os.environ.get("KALLOC", "0")) or KREPEAT


def build_kernel():
    nc = bacc.Bacc("TRN2", target_bir_lowering=False, debug=False, num_devices=1)

    din = {}

    def inp(name, shape, dtype):
        din[name] = nc.dram_tensor(name, list(shape), dtype, kind="ExternalInput")
        return din[name]

    inp("xin", (KALLOC * L, D), F32)
    inp("w_inT", (128, DC * E), BF16)  # ln1-folded in_proj weight, d-major blocks
    inp("c_in", (128, ET), F32)  # in_proj bias column per e-tile (W' @ ln1_b)
    inp("diag_x", (128, D2T * KC * 128), BF16)  # conv diag matrices side by side
    inp("diag_z", (128, D2T * KC * 128), BF16)
    inp("x_projT", (128, D2T * (R + 2 * NS)), F32)
    inp("dt_projT", (R, D2), F32)
    inp("dt_bias", (128, D2T), F32)
    inp("A_perm", (128, NG), F32)  # A[d(p), n(p)] per group column
    inp("rep", (128, 16 * 128), F32)  # delta broadcast matmul: REP[q] blocks
    inp("diag_D", (128, D2T * 128), F32)  # diag(D) per channel tile
    inp("sel", (128, 16 * 128), BF16)  # n-reduction matmul: SEL[q] blocks
    inp("out_projT", (128, ET * D), BF16)
    inp("fc1T", (128, DC * H), BF16)  # ln2-folded fc1 weight
    inp("c_fc1", (128, HT), F32)  # fc1' @ ln2_b + fc1_b per h-tile
    inp("fc2T", (128, HT * D), BF16)
    inp("fc2b", (1, D), F32R)
    inp("ident_bf", (128, 128), BF16)
    inp("zpad", (128, 3), BF16)
    inp("ones1d", (1, 128), F32R)
    inp("rep_b", (2 * NS, 128), BF16)
    inp("rep_c", (2 * NS, 128), BF16)
    inp("ident_f", (128, 128), F32)

    out_d = nc.dram_tensor("out", [KALLOC * L, D], F32, kind="ExternalOutput")

    with tile.TileContext(nc) as tc:
        for rep_i in range(KREPEAT):
            _body(nc, tc, din, out_d, rep_i * L)
    nc.compile()
    return nc


def _body(nc, tc, din, out_d, row0=0):
    xin = din["xin"].ap()[row0 : row0 + L, :]
    out_ap = out_d.ap()[row0 : row0 + L, :]

    with (
        tc.tile_pool(name="pW3", bufs=1) as pW3,  # phase-3 weights, loaded at t=0
        tc.tile_pool(name="p13", bufs=1) as p13,  # crosses into phase 3
        tc.tile_pool(name="p12", bufs=1) as p12,  # dies after phase 2
    ):
        # tiles allocated up front; DMAs for later-phase weights are issued
        # later so the single FIFO DMA queue serves the critical path first
        out_projT = pW3.tile([128, ET * D], BF16, name="opT", tag="opT")
        sel = pW3.tile([128, 16 * 128], BF16, name="sel", tag="sel")
        rep = pW3.tile([128, 16 * 128], F32R, name="rep", tag="rep")
        diag_D = pW3.tile([128, D2T * 128], F32R, name="diagD", tag="diagD")
        c_fc1 = pW3.tile([128, HT], F32, name="cfc1", tag="cfc1")
        fc2b = pW3.tile([1, D], F32R, name="fc2b", tag="fc2b")
        ones1 = pW3.tile([1, 128], F32R, name="ones1", tag="ones1")
        ident_bf = pW3.tile([128, 128], BF16, name="idbf", tag="idbf")
        ident_f = pW3.tile([128, 128], F32, name="idf", tag="idf")
        nc.sync.dma_start(out=ident_f[:, :], in_=din["ident_f"].ap()[:, :])

        # phase1->3 tensors
        zh = [p13.tile([128, L], BF16, name=f"zh{i}", tag=f"zh{i}") for i in range(D2T)]
        y_cm = [p13.tile([128, L], BF16, name=f"ycm{i}", tag=f"ycm{i}") for i in range(D2T)]
        x_res = [p13.tile([128, D], F32, name=f"xres{i}", tag=f"xres{i}") for i in range(TT)]
        # phase1->2 tensors
        xh = [p12.tile([128, L], F32R, name=f"xh{i}", tag=f"xh{i}") for i in range(D2T)]
        delta = [p12.tile([128, L], F32R, name=f"dl{i}", tag=f"dl{i}") for i in range(D2T)]
        du = [p12.tile([128, L], BF16, name=f"du{i}", tag=f"du{i}") for i in range(D2T)]
        bbc = p12.tile([128, L], BF16, name="bbc", tag="bbc")
        cbc = p12.tile([128, L], BF16, name="cbc", tag="cbc")
        xdbl_dt = p12.tile([R, L], F32R, name="xdbl", tag="xdbl")
        a_perm = p12.tile([128, NG], F32, name="aperm", tag="aperm")
        dt_bias = p12.tile([128, D2T], F32, name="dtb", tag="dtb")
        c_in = p12.tile([128, ET], F32, name="cin", tag="cin")

        nc.sync.dma_start(out=c_in[:, :], in_=din["c_in"].ap()[:, :])
        eps_t = p12.tile([128, 1], F32, name="eps_t", tag="eps_t")
        nc.vector.memset(eps_t[:, :], EPS)

        # ================= PHASE 1 ==========
        with (
            tc.tile_pool(name="wE", bufs=1) as wE,
            tc.tile_pool(name="xpP", bufs=1) as xpP,
            tc.tile_pool(name="t1", bufs=2) as t1,
            tc.tile_pool(name="ts", bufs=3) as ts,
            tc.tile_pool(name="tsp", bufs=2) as tsp,
            tc.tile_pool(name="tbc", bufs=1) as tbc,
            tc.tile_pool(name="xhatT_p", bufs=1) as xhatT_p,
        ):
            w_inT = wE.tile([128, DC * E], BF16, name="winT", tag="winT")
            diag = {}
            for br in ("x", "z"):
                diag[br] = wE.tile(
                    [128, D2T * KC * 128], BF16, name=f"diag{br}", tag=f"diag{br}"
                )
            x_projT = wE.tile(
                [128, D2T * (R + 2 * NS)], F32R, name="xpj", tag="xpj"
            )
            dt_projT = wE.tile([R, D2], F32R, name="dtpj", tag="dtpj")
            rep_b = wE.tile([2 * NS, 128], BF16, name="rep_b", tag="rep_b")
            rep_c = wE.tile([2 * NS, 128], BF16, name="rep_c", tag="rep_c")

            xhatT = [xhatT_p.tile([128, L], BF16, name=f"xhT{i}", tag=f"xhT{i}") for i in range(DC)]

            # ---- LN1 (token-major) + transpose ----
            with tc.tile_pool(name="psTr", bufs=2, space="PSUM") as psTr:
                for tt in range(TT):
                    x_t = x_res[tt]
                    nc.sync.dma_start(out=x_t[:, :], in_=xin[tt * 128 : (tt + 1) * 128, :])
                    stats = ts.tile([128, 6], F32, name="stats", tag="stats")
                    nc.vector.bn_stats(out=stats[:, :], in_=x_t[:, :])
                    mv = ts.tile([128, 2], F32, name="mv", tag="mv")
                    nc.vector.bn_aggr(out=mv[:, :], in_=stats[:, :])
                    sd = ts.tile([128, 1], F32, name="sd", tag="sd")
                    nc.scalar.activation(
                        out=sd[:, :], in_=mv[:, 1:2], func=AF.Sqrt, bias=eps_t[:, :], scale=1.0
                    )
                    r_t = ts.tile([128, 1], F32, name="r_t", tag="r_t")
                    nc.vector.reciprocal(out=r_t[:, :], in_=sd[:, :])
                    xhat = t1.tile([128, D], F32, name="xhat", tag="xhat")
                    nc.vector.tensor_scalar(
                        out=xhat[:, :],
                        in0=x_t[:, :],
                        scalar1=mv[:, 0:1],
                        scalar2=r_t[:, :],
                        op0=ALU.subtract,
                        op1=ALU.mult,
                    )
                    ps_tr = psTr.tile([128, D], F32, name="ps_tr", tag="ps_tr")
                    for dc in range(DC):
                        nc.tensor.transpose(
                            ps_tr[:, dc * 128 : (dc + 1) * 128],
                            xhat[:, dc * 128 : (dc + 1) * 128],
                            ident_f[:, :],
                        )
                    for dc in range(DC):
                        nc.vector.tensor_copy(
                            xhatT[dc][:, tt * 128 : (tt + 1) * 128],
                            ps_tr[:, dc * 128 : (dc + 1) * 128],
                        )

            # weight DMAs behind xin on the FIFO queue, ordered by first use
            nc.sync.dma_start(out=w_inT[:, :], in_=din["w_inT"].ap()[:, :])
            nc.sync.dma_start(
                out=diag["x"][:, :], in_=din["diag_x"].ap()[:, :]
            )
            nc.sync.dma_start(out=x_projT[:, :], in_=din["x_projT"].ap()[:, :].bitcast(F32R))
            nc.sync.dma_start(out=dt_projT[:, :], in_=din["dt_projT"].ap()[:, :].bitcast(F32R))
            nc.sync.dma_start(out=rep_b[:, :], in_=din["rep_b"].ap()[:, :])
            nc.sync.dma_start(out=rep_c[:, :], in_=din["rep_c"].ap()[:, :])
            nc.sync.dma_start(out=dt_bias[:, :], in_=din["dt_bias"].ap()[:, :])
            nc.sync.dma_start(
                out=diag["z"][:, :], in_=din["diag_z"].ap()[:, :]
            )
            nc.sync.dma_start(out=rep[:, :], in_=din["rep"].ap()[:, :].bitcast(F32R))
            nc.sync.dma_start(out=sel[:, :], in_=din["sel"].ap()[:, :])
            nc.sync.dma_start(out=a_perm[:, :], in_=din["A_perm"].ap()[:, :])
            nc.sync.dma_start(
                out=diag_D[:, :], in_=din["diag_D"].ap()[:, :].bitcast(F32R)
            )
            nc.sync.dma_start(out=out_projT[:, :], in_=din["out_projT"].ap()[:, :])
            nc.sync.dma_start(out=ident_bf[:, :], in_=din["ident_bf"].ap()[:, :])
            nc.sync.dma_start(out=c_fc1[:, :], in_=din["c_fc1"].ap()[:, :])
            nc.sync.dma_start(out=fc2b[:, :], in_=din["fc2b"].ap()[:, :])
            nc.sync.dma_start(out=ones1[:, :], in_=din["ones1d"].ap()[:, :])

            # ---- conv input buffers (padded by 1 left / 2 right) ----
            xp = {
                "x": [xpP.tile([128, L + 3], BF16, name=f"xpx{i}", tag=f"xpx{i}") for i in range(D2T)],
                "z": [xpP.tile([128, L + 3], BF16, name=f"xpz{i}", tag=f"xpz{i}") for i in range(D2T)],
            }
            for br in ("x", "z"):
                for dt in range(D2T):
                    nc.sync.dma_start(out=xp[br][dt][:, 0:1], in_=din["zpad"].ap()[:, 0:1])
                    nc.sync.dma_start(
                        out=xp[br][dt][:, L + 1 : L + 3], in_=din["zpad"].ap()[:, 0:2]
                    )

            with (
                tc.tile_pool(name="psIn", bufs=2, space="PSUM") as psIn,
                tc.tile_pool(name="psConv", bufs=2, space="PSUM") as psConv,
            ):

                def in_proj(et):
                    ps = psIn.tile([128, L], F32, name="ps_inp", tag="ps_inp")
                    for lc in range(2):
                        for dc in range(DC):
                            nc.tensor.matmul(
                                ps[:, lc * 512 : (lc + 1) * 512],
                                w_inT[:, dc * E + et * 128 : dc * E + (et + 1) * 128],
                                xhatT[dc][:, lc * 512 : (lc + 1) * 512],
                                start=(dc == 0),
                                stop=(dc == DC - 1),
                            )
                    br, dt = ("x", et) if et < D2T else ("z", et - D2T)
                    # Pool cannot read PSUM on HW; x-branch evacs on DVE
                    # (slack in P1), z-branch on ACT (Identity: no table load)
                    nc.vector.tensor_scalar(
                        out=xp[br][dt][:, 1 : 1 + L],
                        in0=ps[:, :],
                        scalar1=c_in[:, et : et + 1],
                        scalar2=None,
                        op0=ALU.add,
                    )

                def conv(br, dt):
                    ps = psConv.tile([128, L], F32, name="ps_conv", tag="ps_conv")
                    for lc in range(2):
                        for j in range(KC):
                            nc.tensor.matmul(
                                ps[:, lc * 512 : (lc + 1) * 512],
                                diag[br][:, (dt * KC + j) * 128 : (dt * KC + j + 1) * 128],
                                xp[br][dt][:, lc * 512 + j : lc * 512 + j + 512],
                                start=(j == 0),
                                stop=(j == KC - 1),
                            )
                    dst = xh[dt][:, :] if br == "x" else zh[dt][:, :]
                    nc.scalar.activation(
                        out=dst, in_=ps[:, :], func=AF.Silu, bias=0.0, scale=1.0
                    )

                # ---- critical x branch: in_proj -> conv -> silu ----
                for et in range(D2T):
                    in_proj(et)
                for dt in range(D2T):
                    conv("x", dt)

                # ---- x_proj: x_dbl[r, l] = x_projT.T @ xh ----
                bc_sb = tbc.tile([2 * NS, L], BF16, name="bc_sb", tag="bc_sb")
                RW = R + 2 * NS
                ps = psIn.tile([128, L], F32, name="ps_xdbl", tag="ps_inp")
                for lc in range(2):
                    for dt in range(D2T):
                        nc.tensor.matmul(
                            ps[0:RW, lc * 512 : (lc + 1) * 512],
                            x_projT[:, dt * RW : (dt + 1) * RW],
                            _f32r(xh[dt][:, lc * 512 : (lc + 1) * 512]),
                            start=(dt == 0),
                            stop=(dt == D2T - 1),
                        )
                nc.vector.tensor_copy(xdbl_dt[:, :], ps[0:R, :])
                nc.vector.tensor_copy(bc_sb[:, :], ps[R : R + 2 * NS, :])

                # broadcast B and C across the 8-channel groups via PE selection
                for dst_t, rep_t in ((bbc, rep_b), (cbc, rep_c)):
                    ps2 = psIn.tile([128, L], F32, name="ps_bc", tag="ps_inp")
                    for lc in range(2):
                        nc.tensor.matmul(
                            ps2[:, lc * 512 : (lc + 1) * 512],
                            rep_t[:, :],
                            bc_sb[:, lc * 512 : (lc + 1) * 512],
                            start=True,
                            stop=True,
                        )
                    nc.vector.tensor_copy(dst_t[:, :], ps2[:, :])

                # ---- dt_proj + softplus -> delta ; du = delta * xh ----
                # Exps batched in pairs before Lns to limit ACT table loads.
                for dt2 in range(0, D2T, 2):
                    t_sps = []
                    for dt in (dt2, dt2 + 1):
                        ps3 = psConv.tile([128, L], F32, name="ps_dt", tag="ps_conv")
                        for lc in range(2):
                            nc.tensor.matmul(
                                ps3[:, lc * 512 : (lc + 1) * 512],
                                _f32r(dt_projT[:, dt * 128 : (dt + 1) * 128]),
                                _f32r(xdbl_dt[:, lc * 512 : (lc + 1) * 512]),
                                start=True,
                                stop=True,
                            )
                        t_sp = tsp.tile(
                            [128, L], F32, name=f"tsp{dt - dt2}", tag=f"tsp{dt - dt2}", bufs=1
                        )
                        nc.scalar.activation(
                            out=t_sp[:, :],
                            in_=ps3[:, :],
                            func=AF.Exp,
                            bias=dt_bias[:, dt : dt + 1],
                            scale=1.0,
                        )
                        t_sps.append(t_sp)
                    for dt in (dt2, dt2 + 1):
                        nc.scalar.activation(
                            out=delta[dt][:, :],
                            in_=t_sps[dt - dt2][:, :],
                            func=AF.Ln,
                            bias=1.0,
                            scale=1.0,
                        )
                        nc.vector.tensor_tensor(
                            out=du[dt][:, :],
                            in0=delta[dt][:, :].bitcast(F32),
                            in1=xh[dt][:, :].bitcast(F32),
                            op=ALU.mult,
                        )

                # ---- deferred z branch (overlaps scan startup) ----
                for et in range(D2T, ET):
                    in_proj(et)
                for dt in range(D2T):
                    conv("z", dt)

        if STOP_AFTER == 1:
            for dt in range(D2T):
                nc.sync.dma_start(
                    out=out_ap[dt * 128 : (dt + 1) * 128, :],
                    in_=delta[dt][:, 0:512].bitcast(F32),
                )
                nc.sync.dma_start(
                    out=out_ap[512 + dt * 128 : 512 + (dt + 1) * 128, :],
                    in_=xh[dt][:, 0:512].bitcast(F32),
                )
            return

        # ================= PHASE 2: selective scan ==========
        # pF outlives phase 2 (fc weights stream in during the scan on the
        # ACT HWDGE queue; its triggers are emitted before the scan exps)
        import contextlib

        _pf_stack = contextlib.ExitStack()
        pF = _pf_stack.enter_context(tc.tile_pool(name="pF", bufs=1))
        fc1T = pF.tile([128, DC * H], BF16, name="fc1T", tag="fc1T")
        nc.scalar.dma_start(out=fc1T[:, :], in_=din["fc1T"].ap()[:, :])
        fc2T = pF.tile([128, HT * D], BF16, name="fc2T", tag="fc2T")
        nc.scalar.dma_start(out=fc2T[:, :], in_=din["fc2T"].ap()[:, :])
        with (
            tc.tile_pool(name="scanp", bufs=4) as scanp,
            tc.tile_pool(name="psDelta", bufs=2, space="PSUM") as psDelta,
            tc.tile_pool(name="psY", bufs=2, space="PSUM") as psY,
        ):
            pair_i = 0
            for dt in range(D2T):
                ps_y = psY.tile([128, L], F32, name="ps_y", tag="ps_y")
                # D*u contribution folded in as a diagonal matmul
                for lc in range(2):
                    nc.tensor.matmul(
                        ps_y[:, lc * 512 : (lc + 1) * 512],
                        diag_D[:, dt * 128 : (dt + 1) * 128],
                        _f32r(xh[dt][:, lc * 512 : (lc + 1) * 512]),
                        start=True,
                        stop=False,
                    )
                for qp in range(8):  # pairs of groups
                    q0 = 2 * qp
                    # paired tiles covering groups q0 and q0+1
                    dA = scanp.tile([128, 2 * L], F32, name="dA", tag="dA", bufs=3)
                    dub = scanp.tile([128, 2 * L], BF16, name="dub", tag="dub")
                    dBu = scanp.tile([128, 2 * L], BF16, name="dBu", tag="dBu")
                    hs = scanp.tile([128, 2 * L], BF16, name="hs", tag="hs", bufs=3)
                    yt = dub  # dub is dead once dBu is built; reuse for yt
                    for h in range(2):
                        q = q0 + h
                        g = dt * 16 + q
                        # delta broadcast via PE: ps_d[p, l] = delta[dt][q*8+p//16, l]
                        ps_d = psDelta.tile([128, L], F32, name="ps_d", tag="ps_d")
                        for lc in range(2):
                            nc.tensor.matmul(
                                ps_d[:, lc * 512 : (lc + 1) * 512],
                                rep[:, q * 128 : (q + 1) * 128],
                                _f32r(delta[dt][:, lc * 512 : (lc + 1) * 512]),
                                start=True,
                                stop=True,
                            )
                        nc.scalar.activation(
                            out=dA[:, h * L : (h + 1) * L],
                            in_=ps_d[:, :],
                            func=AF.Exp,
                            bias=0.0,
                            scale=a_perm[:, g : g + 1],
                        )
                        # du broadcast via SBUF->SBUF DMA
                        nc.sync.dma_start(
                            out=dub[:, h * L : (h + 1) * L],
                            in_=du[dt][q * 8 : (q + 1) * 8, :]
                            .unsqueeze(1)
                            .broadcast_to([8, NS, L]),
                        )
                    # dBu = dub * B on Pool (software TT; keeps DVE for scans);
                    # a few pairs go to DVE to balance
                    dbu_eng = nc.vector if pair_i % 16 == 7 else nc.gpsimd
                    dbu_eng.tensor_tensor(
                        out=dBu[:, :],
                        in0=dub[:, :],
                        in1=bbc[:, :].unsqueeze(1).broadcast_to([128, 2, L]),
                        op=ALU.mult,
                    )
                    pair_i += 1
                    for h in range(2):
                        nc.vector.tensor_tensor_scan(
                            hs[:, h * L : (h + 1) * L],
                            dA[:, h * L : (h + 1) * L],
                            dBu[:, h * L : (h + 1) * L],
                            0.0,
                            ALU.mult,
                            ALU.add,
                        )
                    # yt = hs * C (paired)
                    nc.vector.tensor_tensor(
                        out=yt[:, :],
                        in0=hs[:, :],
                        in1=cbc[:, :].unsqueeze(1).broadcast_to([128, 2, L]),
                        op=ALU.mult,
                    )
                    for h in range(2):
                        q = q0 + h
                        for lc in range(2):
                            nc.tensor.matmul(
                                ps_y[:, lc * 512 : (lc + 1) * 512],
                                sel[:, q * 128 : (q + 1) * 128],
                                yt[:, h * L + lc * 512 : h * L + (lc + 1) * 512],
                                start=False,
                                stop=(q == 15),
                            )
                # evac: y_cm = y_ssm + D*u (already accumulated in psum)
                nc.scalar.copy(out=y_cm[dt][:, :], in_=ps_y[:, :])

        if STOP_AFTER == 2:
            for dt in range(D2T):
                nc.gpsimd.dma_start(
                    out=out_ap[dt * 128 : (dt + 1) * 128, 0:256],
                    in_=y_cm[dt][:, 0:256],
                )
            _pf_stack.close()
            return

        # ================= PHASE 3: out_proj, LN2, MLP ==========
        with (
            tc.tile_pool(name="p3", bufs=1) as p3,
            tc.tile_pool(name="t3", bufs=2) as t3,
            tc.tile_pool(name="psG3", bufs=2, space="PSUM") as psG3,
            tc.tile_pool(name="psF1", bufs=2, space="PSUM") as psF1,
            tc.tile_pool(name="psTr3", bufs=2, space="PSUM") as psTr3,
        ):
            h_res = [p3.tile([128, D], F32, name=f"hres{i}", tag=f"hres{i}") for i in range(TT)]
            xhat2 = [p3.tile([128, D], BF16, name=f"xh2{i}", tag=f"xh2{i}") for i in range(TT)]
            eps3 = p3.tile([128, 1], F32, name="eps3", tag="eps3")
            nc.vector.memset(eps3[:, :], EPS)
            xhat2T = [p3.tile([128, L], BF16, name=f"xh2T{i}", tag=f"xh2T{i}") for i in range(DC)]
            aT = [p3.tile([128, L], BF16, name=f"aT{i}", tag=f"aT{i}") for i in range(HT)]

            # ---- out_proj + residual 1 + LN2 prep ----
            for tt in range(TT):
                x_t = x_res[tt]
                ps = psG3.tile([128, D], F32, name="ps_op", tag="g3")
                korder = list(range(D2T, ET)) + list(range(D2T))
                for ki, k in enumerate(korder):
                    lhs = (
                        y_cm[k][:, tt * 128 : (tt + 1) * 128]
                        if k < D2T
                        else zh[k - D2T][:, tt * 128 : (tt + 1) * 128]
                    )
                    nc.tensor.matmul(
                        ps[:, :],
                        lhs,
                        out_projT[:, k * D : (k + 1) * D],
                        start=(ki == 0),
                        stop=(ki == ET - 1),
                    )
                # + residual on DVE (also evacuates the psum)
                nc.vector.tensor_tensor(
                    out=h_res[tt][:, :], in0=ps[:, :], in1=x_t[:, :], op=ALU.add
                )
                # LN2
                stats = t3.tile([128, 6], F32, name="stats3", tag="stats3")
                nc.vector.bn_stats(out=stats[:, :], in_=h_res[tt][:, :])
                mv = t3.tile([128, 2], F32, name="mv3", tag="mv3")
                nc.vector.bn_aggr(out=mv[:, :], in_=stats[:, :])
                sd = t3.tile([128, 1], F32, name="sd3", tag="sd3")
                nc.scalar.activation(
                    out=sd[:, :], in_=mv[:, 1:2], func=AF.Sqrt, bias=eps3[:, :], scale=1.0
                )
                r_t = t3.tile([128, 1], F32, name="r3", tag="r3")
                nc.vector.reciprocal(out=r_t[:, :], in_=sd[:, :])
                nc.vector.tensor_scalar(
                    out=xhat2[tt][:, :],
                    in0=h_res[tt][:, :],
                    scalar1=mv[:, 0:1],
                    scalar2=r_t[:, :],
                    op0=ALU.subtract,
                    op1=ALU.mult,
                )

            if STOP_AFTER == 21:
                for tt in range(TT):
                    nc.sync.dma_start(
                        out=out_ap[tt * 128 : (tt + 1) * 128, :], in_=h_res[tt][:, :]
                    )
                return

            # ---- MLP pipelined in L-halves: transpose -> fc1+gelu -> fc2 ----
            for half in range(2):
                for dc in range(DC):
                    ps_t = psTr3.tile([128, 512], BF16, name="ps_t3", tag="ps_t3")
                    for b4 in range(4):
                        tt = half * 4 + b4
                        nc.tensor.transpose(
                            ps_t[:, b4 * 128 : (b4 + 1) * 128],
                            xhat2[tt][:, dc * 128 : (dc + 1) * 128],
                            ident_bf[:, :],
                        )
                    nc.scalar.copy(
                        out=xhat2T[dc][:, half * 512 : (half + 1) * 512], in_=ps_t[:, :]
                    )

                for ht in range(HT):
                    ps = psF1.tile([128, 512], F32, name="ps_fc1", tag="f1")
                    for dc in range(DC):
                        nc.tensor.matmul(
                            ps[:, :],
                            fc1T[:, dc * H + ht * 128 : dc * H + (ht + 1) * 128],
                            xhat2T[dc][:, half * 512 : (half + 1) * 512],
                            start=(dc == 0),
                            stop=(dc == DC - 1),
                        )
                    nc.scalar.activation(
                        out=aT[ht][:, half * 512 : (half + 1) * 512],
                        in_=ps[:, :],
                        func=AF.Gelu,
                        bias=c_fc1[:, ht : ht + 1],
                        scale=1.0,
                    )

                for tt in range(half * 4, half * 4 + 4):
                    ps = psG3.tile([128, D], F32, name="ps_fc2", tag="g3")
                    for ht in range(HT):
                        nc.tensor.matmul(
                            ps[:, :],
                            aT[ht][:, tt * 128 : (tt + 1) * 128],
                            fc2T[:, ht * D : (ht + 1) * D],
                            start=(ht == 0),
                            stop=False,
                        )
                    nc.tensor.matmul(
                        ps[:, :], ones1[:, :], fc2b[:, :], start=False, stop=True
                    )
                    o_t = t3.tile([128, D], F32, name="o_t", tag="o_t")
                    nc.vector.tensor_tensor(
                        out=o_t[:, :], in0=ps[:, :], in1=h_res[tt][:, :], op=ALU.add
                    )
                    nc.gpsimd.dma_start(
                        out=out_ap[tt * 128 : (tt + 1) * 128, :], in_=o_t[:, :]
                    )
        _pf_stack.close()


def _mk_repbc(row0):
    m = np.zeros((2 * NS, 128), np.float32)
    p = np.arange(128)
    m[row0 + (p % 16), p] = 1.0
    return m


def prep_inputs(inputs):
    """Host-side weight preprocessing. Returns the shared (non-x) in_map."""
    g = {k: np.asarray(v, dtype=np.float32) for k, v in inputs.items()}

    ln1_w, ln1_b = g["ln1_w"], g["ln1_b"]
    ln2_w, ln2_b = g["ln2_w"], g["ln2_b"]

    w_in = g["in_proj_w"] * ln1_w[None, :]  # [E, D]
    c_in = (g["in_proj_w"] @ ln1_b).astype(np.float32)  # [E]

    fc1 = g["fc1_w"] * ln2_w[None, :]  # [H, D]
    c_fc1 = (g["fc1_w"] @ ln2_b + g["fc1_b"]).astype(np.float32)  # [H]

    A = -np.exp(g["A_log"])  # [D2, NS]

    # REP[q][k, m] = 1 iff k == q*8 + m//16   (delta row broadcast)
    rep = np.zeros((16, 128, 128), np.float32)
    for q in range(16):
        m = np.arange(128)
        rep[q, q * 8 + m // 16, m] = 1.0
    # SEL[q][k, m] = 1 iff m == q*8 + k//16   (sum over n into channel rows)
    sel = np.transpose(rep, (0, 2, 1)).copy()
    # REPA[g][k, m] = A[g*8 + m//16, m%16] iff k == (g%16)*8 + m//16
    # so ps_d = repA[g].T @ delta broadcasts delta rows AND scales by A
    # A_perm[p, g] = A[g*8 + p//16, p%16]
    p = np.arange(128)
    gg = np.arange(NG)
    A_perm = A[(gg[None, :] * 8 + (p // 16)[:, None]), (p % 16)[:, None]].astype(
        np.float32
    )
    diag_D = np.zeros((D2T, 128, 128), np.float32)
    idx128 = np.arange(128)
    for dt in range(D2T):
        diag_D[dt, idx128, idx128] = g["ssm_D"][dt * 128 : (dt + 1) * 128]

    conv_x = g["conv_x_w"][:, 0, :]  # [D2, KC]
    conv_z = g["conv_z_w"][:, 0, :]
    diag_x = np.zeros((D2T * KC, 128, 128), np.float32)
    diag_z = np.zeros((D2T * KC, 128, 128), np.float32)
    idx = np.arange(128)
    for dt in range(D2T):
        for j in range(KC):
            diag_x[dt * KC + j, idx, idx] = conv_x[dt * 128 : (dt + 1) * 128, j]
            diag_z[dt * KC + j, idx, idx] = conv_z[dt * 128 : (dt + 1) * 128, j]

    def bf(x):
        return np.ascontiguousarray(x.astype(_BF))

    def f(x):
        return np.ascontiguousarray(x.astype(np.float32))

    def blocks(arr3):  # [N, 128, W] -> [128, N*W]
        n, pdim, w = arr3.shape
        assert pdim == 128
        return np.ascontiguousarray(arr3.transpose(1, 0, 2).reshape(128, n * w))

    RW = R + 2 * NS
    shared = {
        "w_inT": bf(blocks(w_in.T.reshape(DC, 128, E))),
        "c_in": f(c_in.reshape(ET, 128).T),
        "diag_x": bf(blocks(diag_x)),
        "diag_z": bf(blocks(diag_z)),
        "x_projT": f(blocks(g["x_proj_w"].T.reshape(D2T, 128, RW))),
        "dt_projT": f(g["dt_proj_w"].T),
        "dt_bias": f(g["dt_proj_b"].reshape(D2T, 128).T),
        "A_perm": f(A_perm),
        "rep": f(blocks(rep)),
        "diag_D": f(blocks(diag_D)),
        "sel": bf(blocks(sel)),
        "out_projT": bf(blocks(g["out_proj_w"].T.reshape(ET, 128, D))),
        "fc1T": bf(blocks(fc1.T.reshape(DC, 128, H))),
        "c_fc1": f(c_fc1.reshape(HT, 128).T),
        "fc2T": bf(blocks(g["fc2_w"].T.reshape(HT, 128, D))),
        "fc2b": f(g["fc2_b"].reshape(1, D)),
        "ident_bf": bf(np.eye(128, dtype=np.float32)),
        "zpad": np.zeros((128, 3), _BF),
        "ones1d": np.ones((1, 128), np.float32),
        "rep_b": bf(_mk_repbc(0)),
        "rep_c": bf(_mk_repbc(NS)),
        "ident_f": np.eye(128, dtype=np.float32),
    }
    return shared


_CACHED_NC = None


def kernel(**inputs):
    global _CACHED_NC
    from concourse.bass_utils import run_bass_kernel_spmd

    if _CACHED_NC is None:
        _CACHED_NC = build_kernel()
    nc = _CACHED_NC

    shared = prep_inputs(inputs)
    x = np.asarray(inputs["x"], dtype=np.float32)
    in_maps = [
        dict(
            shared,
            xin=np.ascontiguousarray(np.concatenate([x[i]] * KREPEAT, axis=0)),
        )
        for i in range(NCORES)
    ]
    res = run_bass_kernel_spmd(nc, in_maps, core_ids=list(range(NCORES)))
    out = np.stack([res.results[i]["out"][:L] for i in range(NCORES)], axis=0)
    return out


if __name__ == "__main__":
    nc = build_kernel()
    print("build ok")
